# revision 1
# baseline (speedup 1.0000x reference)
"""Trainium2 Bass kernel for nn_ContextKGEModel (self-attentive path pooling + FFN hinge loss).

Data-parallel over the 2048 ragged groups, 8 NeuronCores:
  - Host: assign 16 whole batch rows per core (load-balanced), first-fit-
    decreasing-pack each core's 256 groups into 128-row bins, and ship
    triple_emb in two fp8-e4m3 layouts (row-major bins with an appended ones
    column + group-mask block, and a transposed copy in supertiles of 4 bins).
    Weights are replicated and pre-transposed; W1/b1 are host-scaled by 8 and
    W2 by 16 so they stay in fp8 normal range (the 1/128 folds into the
    sigmoid scale). A +/-1 pair-selection matrix encodes the hinge pairs.
  - Device (per core): xwT = W_sfa^T @ X^T per supertile and the per-bin Gram
    xw X^T run as fp8 DoubleRow matmuls; the group-masked column max is taken
    on the raw Gram (tanh is monotone so it commutes with max), then tiny
    tanh+exp; attention weights are built by an iota-vs-slot compare fused
    with the exp scale; unnormalized pooled vectors accumulate in PSUM across
    all bins (the ones column accumulates the softmax denominator, one
    reciprocal normalizes at the end); the FFN runs fp8 DoubleRow; the hinge
    loss is computed on-chip via the pair-selection matmul. The xw, attention,
    and pooled pipelines are software-pipelined 1-2 supertiles apart so PE
    never waits on the ACT/DVE softmax chain. Host sums the 8 partial losses.
"""

import os
import threading
from contextlib import ExitStack

import numpy as np
import ml_dtypes

import concourse.bass as bass
import concourse.tile as tile
from concourse import mybir
from concourse.vector_clock import ScopedClock
from concourse.bass_utils import run_bass_kernel_spmd
from concourse.masks import make_identity

bf16 = ml_dtypes.bfloat16
fp8 = ml_dtypes.float8_e5m2
fp8e4 = ml_dtypes.float8_e4m3

B, NEG, L, D = 128, 15, 32, 768
NPAIR_SET = 120                      # 240 hinge pairs split into 2 matmul sets
G = B * (NEG + 1)
GAMMA = 0.1
NCORES = 8
ROWS_PER_CORE = B // NCORES          # 16 batch rows / core
SLOTS = ROWS_PER_CORE * (NEG + 1)    # 256 group slots / core
BIN = 128
KC = D // 128                        # 6 contraction chunks
HC = (4 * D) // 128                  # 24 hidden chunks
DW = D + 8                           # x row + ones column + pad
NEG_MASK = -240.0

_compile_cache = {}
_compile_lock = threading.Lock()


def _patch_tile_drain():
    """This walrus build rejects >1 sem-wait on an instruction ("Too many sync
    wait commands"); split the TileContext tail-drain waits across SP nops."""
    if getattr(tile.TileContext, "_drain_patch_applied", False):
        return

    def _drain_and_barrier(self, tick_clock, wait_clock):
        probe = self.nc.sync.nop(nofuse=True, hint="drain_wait_split")
        wait_clock.add_sem_waits(probe.ins, ScopedClock({None: tick_clock.global_clock}))
        si = probe.ins.sync_info
        waits = list(si.on_wait) if si is not None and si.on_wait else []
        if len(waits) > 1:
            si.on_wait = waits[:1]
            for w in waits[1:]:
                extra = self.nc.sync.nop(nofuse=True, hint="drain_wait_split")
                esi = extra.ins.sync_info
                if esi is None:
                    extra.ins.sync_info = mybir.SyncInfo(on_wait=[w], on_update=[])
                else:
                    esi.on_wait = [w]
        self.nc.sync.drain()
        self.nc.all_engine_barrier()
        assert self.sems is not None
        popped = self.nc._tile_sem_poison_stack.pop()
        assert popped is self._sem_poison
        self.nc.clear_and_free_semaphores(list(self.sems.allocated().values()))
        self.nc.all_engine_barrier()

    tile.TileContext._drain_and_barrier = _drain_and_barrier
    tile.TileContext._drain_patch_applied = True


_MAX_WAITS = 1


def _split_waits(nc, maxw=_MAX_WAITS):
    """Hoist excess sync-waits onto NoOps inserted just before the
    instruction on the same engine (walrus build caps waits/instruction)."""
    n_split = 0
    for fn in nc.m.functions:
        for bb in fn.blocks:
            out = []
            for inst in bb.instructions:
                si = inst.sync_info
                waits = list(si.on_wait) if si is not None and si.on_wait else []
                if len(waits) > maxw:
                    keep = waits[:maxw]
                    rest = waits[maxw:]
                    for i in range(0, len(rest), maxw):
                        n_split += 1
                        nop = mybir.InstNoOp(
                            name=f"WSPLIT-{n_split}",
                            engine=inst.engine,
                            debug=inst.debug,
                            ins=[], outs=[],
                            sync_info=mybir.SyncInfo(
                                on_wait=rest[i:i + maxw], on_update=[]),
                        )
                        out.append(nop)
                    si.on_wait = keep
                out.append(inst)
            if n_split:
                bb.instructions[:] = out
    return n_split


# ---------------------------------------------------------------- host packing

def _pack(sizes_flat):
    """Balanced batch-row -> core assignment + first-fit-decreasing bin packing."""
    sizes = sizes_flat.reshape(B, NEG + 1)
    row_load = sizes.sum(1)
    order = np.argsort(-row_load, kind="stable")
    core_rows = [[] for _ in range(NCORES)]
    core_load = np.zeros(NCORES, np.int64)
    for b in order:
        cands = [c for c in range(NCORES) if len(core_rows[c]) < ROWS_PER_CORE]
        c = min(cands, key=lambda c: core_load[c])
        core_rows[c].append(int(b))
        core_load[c] += row_load[b]
    bins_all = []
    for c in range(NCORES):
        groups = []
        for lb, b in enumerate(core_rows[c]):
            for k in range(NEG + 1):
                g = b * (NEG + 1) + k
                groups.append((g, lb * (NEG + 1) + k, int(sizes_flat[g])))
        groups.sort(key=lambda t: -t[2])
        bins = []
        for g, slot, n in groups:
            for bn in bins:
                if bn[0] + n <= BIN:
                    bn[1].append((g, slot, n, bn[0]))
                    bn[0] += n
                    break
            else:
                bins.append([n, [(g, slot, n, 0)]])
        bins_all.append([bn[1] for bn in bins])
    return core_rows, bins_all


def _build_core_arrays(bins_c, triple_emb_bf, offsets, NB):
    """Per-core packed device inputs (supertile-major layouts)."""
    NS = NB // 4
    X = np.zeros((NB, BIN, DW), fp8e4)
    X[:, :, D] = 0  # ones col set below only for valid rows (any row is fine)
    gid = np.full((NB, BIN), -1, np.int32)
    slot_of = np.full((NB, BIN), -1, np.int32)
    for bi, bn in enumerate(bins_c):
        for g, slot, n, off in bn:
            X[bi, off:off + n, :D] = triple_emb_bf[offsets[g]:offsets[g] + n].astype(fp8e4)
            gid[bi, off:off + n] = g
            slot_of[bi, off:off + n] = slot
    X[:, :, D] = 1.0  # ones column (padding rows are zeroed via Ind anyway)
    same = (gid[:, :, None] == gid[:, None, :]) & (gid[:, :, None] >= 0)
    m_add = np.where(same, np.float32(0.0), np.float32(NEG_MASK)).astype(fp8e4)
    # supertile-major packings; x + mask merged into one DMA per supertile
    x_st = X.reshape(NS, 4, BIN, DW).transpose(0, 2, 1, 3).reshape(NS, BIN, 4 * DW)
    madd_st = m_add.reshape(NS, 4, BIN, BIN).transpose(0, 2, 1, 3) \
                   .reshape(NS, BIN, 4 * BIN)
    xm = np.ascontiguousarray(np.concatenate([x_st, madd_st], axis=2))
    xt = np.ascontiguousarray(
        X[:, :, :D].reshape(NS, 4, BIN, KC, 128)   # [s, b4, r, c, d]
                   .transpose(0, 4, 3, 1, 2)       # [s, d, c, b4, r]
                   .reshape(NS, 128, KC, 4 * BIN))
    slot_st = np.ascontiguousarray(
        slot_of.astype(np.float32).reshape(NS, 4, BIN).transpose(2, 0, 1))  # [BIN,NS,4]
    return xm, xt, slot_st


# ---------------------------------------------------------------- device program

DEBUG_OUTPUTS = False


def _build_program(NB):
    NS = NB // 4
    nc = bass.Bass()
    dt = mybir.dt
    AF = mybir.ActivationFunctionType

    XMW = 4 * DW + 4 * BIN  # x rows + mask columns, fp8 bytes per partition
    x_d = nc.dram_tensor("x_bins", [NS, BIN, XMW], dt.float8e4, kind="ExternalInput")
    xt_d = nc.dram_tensor("xt_bins", [NS, 128, KC, 4 * BIN], dt.float8e4, kind="ExternalInput")
    slot_d = nc.dram_tensor("slot_of", [BIN, NS, 4], dt.float32, kind="ExternalInput")
    wsfa_d = nc.dram_tensor("w_sfa_t", [128, KC * D], dt.float8e4, kind="ExternalInput")
    w1t_d = nc.dram_tensor("w1_t", [128, KC * 4 * D], dt.float8e4, kind="ExternalInput")
    w2t_d = nc.dram_tensor("w2_t", [128, HC], dt.float8e4, kind="ExternalInput")
    b1_d = nc.dram_tensor("b1_r", [128, HC], dt.float32, kind="ExternalInput")
    b2_d = nc.dram_tensor("b2_r", [1, 1], dt.float32, kind="ExternalInput")
    pair_d = nc.dram_tensor("pair_m", [128, 2, 2, NPAIR_SET], dt.float32,
                            kind="ExternalInput")
    loss_d = nc.dram_tensor("loss", [1, 1], dt.float32, kind="ExternalOutput")
    if DEBUG_OUTPUTS:
        dbg_scores_d = nc.dram_tensor("dbg_scores", [1, SLOTS], dt.float32,
                                      kind="ExternalOutput")
        dbg_exp_d = nc.dram_tensor("dbg_exp", [NB, 128, 1], dt.float32,
                                   kind="ExternalOutput")
        dbg_colmax_d = nc.dram_tensor("dbg_colmax", [NB, 128, 1], dt.float32,
                                      kind="ExternalOutput")
        dbg_pooled_d = nc.dram_tensor("dbg_pooled", [2, 128, D], dt.bfloat16,
                                      kind="ExternalOutput")

    with tile.TileContext(nc) as tc, ExitStack() as ctx:
        consts = ctx.enter_context(tc.tile_pool(name="consts", bufs=1))
        xres = ctx.enter_context(tc.tile_pool(name="xres", bufs=1))
        attres = ctx.enter_context(tc.tile_pool(name="attres", bufs=1))
        xt_pool = ctx.enter_context(tc.tile_pool(name="xt", bufs=4))
        xwt_pool = ctx.enter_context(tc.tile_pool(name="xwt", bufs=4))
        mask_pool = ctx.enter_context(tc.tile_pool(name="masks", bufs=4))
        small = ctx.enter_context(tc.tile_pool(name="small", bufs=12))
        gm_pool = ctx.enter_context(tc.tile_pool(name="gm", bufs=6))
        ffn_pool = ctx.enter_context(tc.tile_pool(name="ffn", bufs=1))

        # resident constants
        wsfa = consts.tile([128, KC, D], dt.float8e4)      # [d_in_chunk, kc, e]
        nc.sync.dma_start(out=wsfa, in_=wsfa_d[:, :].rearrange("p (k e) -> p k e", k=KC))
        slot_all = consts.tile([128, NS, 4], dt.float32)
        nc.sync.dma_start(out=slot_all, in_=slot_d[:, :, :])
        ident = consts.tile([128, 128], dt.bfloat16)
        make_identity(nc, ident)
        iota_i = consts.tile([128, SLOTS], dt.int32)
        nc.gpsimd.iota(iota_i, pattern=[[1, SLOTS]], base=0, channel_multiplier=0)
        iota_f = consts.tile([128, SLOTS], dt.float32)
        nc.vector.tensor_copy(iota_f, iota_i)

        x_tiles = [xres.tile([128, XMW], dt.float8e4, tag=f"x{s}", name=f"x{s}")
                   for s in range(NS)]
        att_pairs = [attres.tile([128, 2, SLOTS], dt.float8e4, tag=f"a{p}", name=f"a{p}")
                     for p in range(NB // 2)]

        # ---- phase A: xwT per supertile; per-bin attention weights one
        # supertile behind; pooled accumulation two supertiles behind
        # (keeps PE off the ACT/DVE softmax critical path)
        with (
            tc.tile_pool(name="ps_xw", bufs=2, space="PSUM") as ps_xw,
            tc.tile_pool(name="ps_gm", bufs=2, space="PSUM") as ps_gm,
            tc.tile_pool(name="ps_pool", bufs=1, space="PSUM") as ps_pooled,
        ):
            xt_tiles = {}
            xwt_tiles = {}

            def emit_load(s):
                xt_t = xt_pool.tile([128, KC, 4 * BIN], dt.float8e4, tag="xt",
                                    name=f"xt{s}")
                nc.sync.dma_start(out=xt_t, in_=xt_d[s])
                nc.sync.dma_start(out=x_tiles[s], in_=x_d[s])
                xt_tiles[s] = xt_t

            def emit_xw(s):
                xt_t = xt_tiles[s]
                xwt_t = xwt_pool.tile([128, KC, 4 * BIN], dt.float8e4, tag="xwt",
                                      name=f"xwt{s}")
                for e in range(KC):
                    ps = ps_xw.tile([128, 4 * BIN], dt.float32, tag="psxw",
                                    name=f"psxw{s}_{e}")
                    for k in range(0, KC, 2):
                        nc.tensor.matmul(
                            ps, wsfa[:, k:k + 2, e * 128:(e + 1) * 128],
                            xt_t[:, k:k + 2, :],
                            start=(k == 0), stop=(k == KC - 2),
                            perf_mode=mybir.MatmulPerfMode.DoubleRow)
                    if e >= 5:
                        nc.vector.tensor_copy(xwt_t[:, e, :], ps)
                    else:
                        nc.scalar.copy(xwt_t[:, e, :], ps)
                xwt_tiles[s] = xwt_t

            def emit_bins(s):
                xt_t, xwt_t = xt_tiles[s], xwt_tiles[s]
                madd_t = x_tiles[s][:, 4 * DW:].rearrange("p (j i) -> p j i", i=BIN)
                slot_t = slot_all[:, s, :]
                ps_g4 = ps_gm.tile([128, 4, BIN], dt.float32, tag="psgm",
                                   name=f"psgm{s}")
                for bp in range(2):
                    pi = s * 2 + bp
                    ps_g = ps_g4[:, 2 * bp:2 * bp + 2, :]
                    for j in range(2):
                        bi = 2 * pi + j
                        sl = slice((2 * bp + j) * BIN, (2 * bp + j + 1) * BIN)
                        for e in range(0, KC, 2):
                            nc.tensor.matmul(ps_g[:, j, :], xwt_t[:, e:e + 2, sl],
                                             xt_t[:, e:e + 2, sl],
                                             start=(e == 0), stop=(e == KC - 2),
                                             perf_mode=mybir.MatmulPerfMode.DoubleRow)
                    gm_m = gm_pool.tile([128, 2, BIN], dt.float32, tag="gmm",
                                        name=f"gmm{pi}")
                    colmax2 = small.tile([128, 2], dt.float32, tag="colmax",
                                         name=f"colmax{pi}")
                    # masked max of raw Gram; tanh applied after the max
                    # (tanh is monotone, so max commutes with it)
                    nc.vector.tensor_add(gm_m, ps_g4[:, 2 * bp:2 * bp + 2, :],
                                         madd_t[:, 2 * bp:2 * bp + 2, :])
                    nc.vector.tensor_reduce(out=colmax2, in_=gm_m,
                                            op=mybir.AluOpType.max,
                                            axis=mybir.AxisListType.X)
                    th2 = small.tile([128, 2], dt.float32, tag="th2",
                                     name=f"th{pi}")
                    nc.scalar.activation(th2, colmax2, AF.Tanh)
                    expv2 = small.tile([128, 2], dt.float32, tag="expv",
                                       name=f"expv{pi}")
                    nc.scalar.activation(expv2, th2, AF.Exp)
                    for j in range(2):
                        nc.vector.tensor_scalar(
                            out=att_pairs[pi][:, j, :], in0=iota_f,
                            scalar1=slot_t[:, 2 * bp + j:2 * bp + j + 1],
                            scalar2=expv2[:, j:j + 1],
                            op0=mybir.AluOpType.is_equal, op1=mybir.AluOpType.mult)
                    if DEBUG_OUTPUTS:
                        for j in range(2):
                            nc.sync.dma_start(out=dbg_exp_d[2 * pi + j],
                                              in_=expv2[:, j:j + 1])
                            nc.sync.dma_start(out=dbg_colmax_d[2 * pi + j],
                                              in_=th2[:, j:j + 1])

            ps_p = [ps_pooled.tile([128, DW], dt.float32, tag=f"psp{h}", name=f"psp{h}")
                    for h in range(2)]
            NP = NB // 2

            def emit_pooled(s):
                for bp in range(2):
                    pi = s * 2 + bp
                    xv = x_tiles[s][:, :4 * DW].rearrange("p (b w) -> p b w", w=DW)
                    for h in range(2):
                        hsl = slice(h * 128, (h + 1) * 128)
                        # keep each matmul output inside one PSUM bank
                        for n0, nlen in ((0, 512), (512, DW - 512)):
                            nc.tensor.matmul(
                                ps_p[h][:, n0:n0 + nlen],
                                att_pairs[pi][:, :, hsl],
                                xv[:, 2 * bp:2 * bp + 2, n0:n0 + nlen],
                                start=(pi == 0), stop=(pi == NP - 1),
                                perf_mode=mybir.MatmulPerfMode.DoubleRow)

            emit_load(0)
            emit_load(1)
            for s in range(NS):
                emit_xw(s)
                if s + 2 < NS:
                    emit_load(s + 2)
                if s >= 1:
                    emit_bins(s - 1)
                if s >= 2:
                    emit_pooled(s - 2)
            emit_bins(NS - 1)
            emit_pooled(NS - 2)
            emit_pooled(NS - 1)

        # FFN weights loaded late so they don't delay the phase-A DMA stream
        w1t = consts.tile([128, KC, 4 * D], dt.float8e4)
        nc.sync.dma_start(out=w1t, in_=w1t_d[:, :].rearrange("p (k h) -> p k h", k=KC))
        w2t = consts.tile([128, HC], dt.float8e4)
        nc.sync.dma_start(out=w2t, in_=w2t_d[:, :])
        b1s = consts.tile([128, HC], dt.float32)
        nc.sync.dma_start(out=b1s, in_=b1_d[:, :])
        b2s = consts.tile([1, 1], dt.float32)
        nc.sync.dma_start(out=b2s, in_=b2_d[:, :])
        pairm = consts.tile([128, 2, 2, NPAIR_SET], dt.float32)
        nc.sync.dma_start(out=pairm, in_=pair_d[:, :, :, :])

        # ---- phase B1: normalize pooled by the accumulated denominator
        pooled_sb = ffn_pool.tile([128, 2, D], dt.bfloat16, tag="pooled")
        if True:
            for h in range(2):
                rz = small.tile([128, 1], dt.float32, tag="rz", name=f"rz{h}")
                nc.vector.reciprocal(rz, ps_p[h][:, D:D + 1])
                nc.vector.tensor_scalar_mul(pooled_sb[:, h, :], ps_p[h][:, :D], rz)
                if DEBUG_OUTPUTS:
                    nc.sync.dma_start(out=dbg_pooled_d[h], in_=pooled_sb[:, h, :])

        # ---- phase B2: transpose pooled, FFN, hinge loss
        with (
            tc.tile_pool(name="ps_t", bufs=2, space="PSUM") as ps_t,
            tc.tile_pool(name="ps_h", bufs=2, space="PSUM") as ps_h,
            tc.tile_pool(name="ps_sc", bufs=1, space="PSUM") as ps_sc,
            tc.tile_pool(name="dram", bufs=1, space="DRAM") as dram_pool,
        ):
            pooledT = ffn_pool.tile([128, KC, SLOTS], dt.float8e4, tag="pooledT")
            for h in range(2):
                for k in range(KC):
                    ps_tr = ps_t.tile([128, 128], dt.bfloat16, tag="pstr",
                                      name=f"pstr{h}_{k}")
                    nc.tensor.transpose(
                        ps_tr, pooled_sb[:, h, k * 128:(k + 1) * 128], ident)
                    if k % 2 == 0:
                        nc.scalar.copy(pooledT[:, k, h * 128:(h + 1) * 128], ps_tr)
                    else:
                        nc.vector.tensor_copy(pooledT[:, k, h * 128:(h + 1) * 128], ps_tr)
            hrelu = ffn_pool.tile([128, HC, SLOTS], dt.float8e4, tag="hrelu")
            for hc in range(HC):
                ps_hh = ps_h.tile([128, SLOTS], dt.float32, tag="psh",
                                  name=f"psh{hc}")
                for k in range(0, KC, 2):
                    nc.tensor.matmul(ps_hh,
                                     w1t[:, k:k + 2, hc * 128:(hc + 1) * 128],
                                     pooledT[:, k:k + 2, :],
                                     start=(k == 0), stop=(k == KC - 2),
                                     perf_mode=mybir.MatmulPerfMode.DoubleRow)
                # W1,b1 host-scaled by 8: hrelu holds 8*h; 1/8 folded into
                # the sigmoid scale below
                nc.vector.tensor_scalar(
                    out=hrelu[:, hc, :], in0=ps_hh, scalar1=b1s[:, hc:hc + 1],
                    scalar2=0.0, op0=mybir.AluOpType.add,
                    op1=mybir.AluOpType.max)
            ps_s = ps_sc.tile([1, SLOTS], dt.float32, tag="ps_s", name="ps_s")
            for hc in range(HC):
                nc.tensor.matmul(ps_s, w2t[:, hc:hc + 1], hrelu[:, hc, :],
                                 start=(hc == 0), stop=(hc == HC - 1))
            scores = ffn_pool.tile([1, SLOTS], dt.float32, tag="scores")
            # W2 x16, W1/b1 x8 host scalings: sigmoid(psum/128 + b2)
            nc.scalar.activation(scores, ps_s, AF.Sigmoid, bias=b2s,
                                 scale=0.0078125)
            if DEBUG_OUTPUTS:
                nc.sync.dma_start(out=dbg_scores_d[:, :], in_=scores[0:1, :])
            # hinge: transpose scores to slot-partition vectors, pair-difference
            # matmuls against the host-built +/-1 selection matrix, relu(+gamma),
            # then a ones-matmul partition sum -- all on-chip
            identf = consts.tile([128, 128], dt.float32)
            make_identity(nc, identf)
            sT = ffn_pool.tile([128, 2], dt.float32, tag="sT")
            for ch in range(2):
                ps_tr2 = ps_t.tile([128, 1], dt.float32, tag="pstr",
                                   name=f"sctr{ch}")
                nc.tensor.transpose(ps_tr2, scores[0:1, ch * 128:(ch + 1) * 128],
                                    identf[0:1, 0:1])
                nc.vector.tensor_copy(sT[:, ch:ch + 1], ps_tr2)
            ps_d = ps_sc.tile([NPAIR_SET, 2], dt.float32, tag="ps_d", name="ps_d")
            for st in range(2):
                for ch in range(2):
                    nc.tensor.matmul(ps_d[:, st:st + 1],
                                     pairm[:, st, ch, :], sT[:, ch:ch + 1],
                                     start=(ch == 0), stop=(ch == 1))
            relu_d = ffn_pool.tile([NPAIR_SET, 2], dt.float32, tag="relu_d")
            nc.vector.tensor_scalar(out=relu_d, in0=ps_d, scalar1=GAMMA,
                                    scalar2=0.0, op0=mybir.AluOpType.add,
                                    op1=mybir.AluOpType.max)
            ones_t = consts.tile([NPAIR_SET, 1], dt.float32)
            nc.vector.memset(ones_t, 1.0)
            ps_l = ps_sc.tile([1, 1], dt.float32, tag="ps_l", name="ps_l")
            for st in range(2):
                nc.tensor.matmul(ps_l, relu_d[:, st:st + 1], ones_t,
                                 start=(st == 0), stop=(st == 1))
            loss_sb = ffn_pool.tile([1, 1], dt.float32, tag="loss")
            nc.scalar.activation(loss_sb, ps_l, AF.Copy)
            nc.sync.dma_start(out=loss_d[:, :], in_=loss_sb)

    _split_waits(nc)
    return nc


# ---------------------------------------------------------------- entry point

def kernel(triple_emb, W_sfa, W1, b1, W2, b2, tri2path_size):
    _patch_tile_drain()
    triple_emb = np.asarray(triple_emb, np.float32)
    sizes_flat = np.asarray(tri2path_size, np.int32).reshape(-1).astype(np.int64)
    offsets = np.concatenate([[0], np.cumsum(sizes_flat)[:-1]])

    core_rows, bins_all = _pack(sizes_flat)
    NB = max(len(b) for b in bins_all)
    NB = ((NB + 3) // 4) * 4

    triple_bf = triple_emb.astype(bf16)
    wsfa_t = np.ascontiguousarray(
        np.asarray(W_sfa, np.float32).T.reshape(KC, 128, D).transpose(1, 0, 2)
        .reshape(128, KC * D)).astype(fp8e4)
    w1_t = np.ascontiguousarray(
        (np.asarray(W1, np.float32) * 8.0).T.reshape(KC, 128, 4 * D)
        .transpose(1, 0, 2).reshape(128, KC * 4 * D)).astype(fp8e4)
    w2_t = np.ascontiguousarray(
        (np.asarray(W2, np.float32) * 16.0).reshape(HC, 128).T).astype(fp8e4)
    b1_r = np.ascontiguousarray(
        (np.asarray(b1, np.float32) * 8.0).reshape(HC, 128).T)
    b2_r = np.asarray(b2, np.float32).reshape(1, 1)
    pair_m = np.zeros((128, 2, 2, NPAIR_SET), np.float32)
    for t in range(ROWS_PER_CORE * NEG):
        st, j = divmod(t, NPAIR_SET)
        b, k = divmod(t, NEG)
        slot_n = 16 * b + (k + 1)
        slot_p = 16 * b
        pair_m[slot_n % 128, st, slot_n // 128, j] += 1.0
        pair_m[slot_p % 128, st, slot_p // 128, j] -= 1.0

    in_maps = []
    for c in range(NCORES):
        xm, xt, slot_st = _build_core_arrays(bins_all[c], triple_bf, offsets, NB)
        in_maps.append({
            "x_bins": xm, "xt_bins": xt, "slot_of": slot_st,
            "w_sfa_t": wsfa_t, "w1_t": w1_t, "w2_t": w2_t,
            "b1_r": b1_r, "b2_r": b2_r, "pair_m": pair_m,
        })

    with _compile_lock:
        nc = _compile_cache.get(NB)
        if nc is None:
            nc = _build_program(NB)
            _compile_cache[NB] = nc

    res = run_bass_kernel_spmd(nc, in_maps, core_ids=list(range(NCORES)),
                               trace=bool(int(os.environ.get("KGE_TRACE", "0"))))
    total = np.float64(0.0)
    for r in res.results:
        total += np.float64(r["loss"][0, 0])
    kernel.last_results = res
    return np.asarray(np.float32(total))

